# revision 37
# baseline (speedup 1.0000x reference)
"""Trainium2 Bass kernel for nn_CMoSModel (moe_routing).

Data-parallel over batch: bs=256 -> 32 per core on 8 cores; params replicated.

Math (reference):
  xt = x.T(b,c,L); mean/std over L; xn = (xt-mean)/std
  conv = depthwise_conv1d(xn, k=16, stride=8) + conv_b       [b,c,63]
  gates = softmax(conv @ gate_w.T + gate_b); top2 -> softmax(vals) -> dense g
  y = einsum('bcsn,mon->bcmos', xn.resh, map_w) + map_b; out = (y.comb)*std+mean

Key identities exploited:
  * std cancels in the expert path: W(xn)*std = W(xc), so experts run on
    xc scaled by raw gate g (not g*std).  Bias term = (sum_m g_m b_m)*std
    + mean, applied via a small rank-9 matmul with gsm=(g*std | mean).
  * conv_bias folds into the gate bias: logits = (conv_raw^T diag(rstd))^T
    @ gate_w^T + (conv_b*rowsum(gate_w) + gate_b).

Layout strategy (per 128-row tile r=(h,c), 2 batches x 64 ch):
  * gate-scaled transposes via PE matmul against diag(g_m): xgt_m =
    xc_block^T @ D_m -- no explicit scaled copies, no strip-move DMAs.
  * expert matmuls flipped (W stationary, activations moving) with
    block-diagonal 2-pair packing: lhsT = blkdiag(w_m, w_m) [64, 90],
    rhs = xgt_m[64*pg:64*(pg+1), (q,r)=512].  Output lands transposed
    [(sp,o), (q, h, c)] and is DMA'd straight out -- no output transposes.
"""

import os
import sys

import numpy as np

for p in ("/opt/trn_rl_repo", "/opt/pypackages"):
    if p not in sys.path:
        sys.path.insert(0, p)

BS = 256
SEQ = 512
PRED = 720
C = 64
SEG = 16
NM = 8
KSZ = 16
STRIDE = 8
CONV_DIM = 63
N_IN = 32
N_OUT = 45
NCORES = 8
BPC = BS // NCORES   # 32 batches per core
NT = BPC // 2        # 16 tiles, 2 batches each (128 rows of (h,c))
G = 4                # tiles per conv group

_CACHE = {}


def _build_program(mm_dt_name="bfloat16"):
    import concourse.bass as bass
    import concourse.tile as tile
    from concourse import bacc
    from concourse import mybir
    from concourse.masks import make_identity

    f32 = mybir.dt.float32
    mm_dt = getattr(mybir.dt, mm_dt_name)
    AL = mybir.AluOpType
    AF = mybir.ActivationFunctionType
    AX = mybir.AxisListType

    nc = bacc.Bacc(None, target_bir_lowering=False)
    x_d = nc.declare_dram_parameter("x", [BPC, SEQ, C], f32, isOutput=False)
    cw_d = nc.declare_dram_parameter("conv_w", [C, 1, KSZ], f32, isOutput=False)
    cb_d = nc.declare_dram_parameter("conv_b", [C], f32, isOutput=False)
    gw_d = nc.declare_dram_parameter("gate_w", [NM, CONV_DIM], f32, isOutput=False)
    gb_d = nc.declare_dram_parameter("gate_b", [NM], f32, isOutput=False)
    mw_d = nc.declare_dram_parameter("map_w", [NM, N_OUT, N_IN], f32, isOutput=False)
    mb_d = nc.declare_dram_parameter("map_b", [NM, N_OUT], f32, isOutput=False)
    out_d = nc.declare_dram_parameter("out", [BPC, PRED, C], f32, isOutput=True)

    inv_L = 1.0 / SEQ

    with tile.TileContext(nc) as tc:
        with (
            tc.tile_pool(name="consts", bufs=1) as consts,
            tc.tile_pool(name="xin", bufs=3) as xin,
            tc.tile_pool(name="xcg", bufs=2) as xcg,
            tc.tile_pool(name="cvg", bufs=2) as cvg,
            tc.tile_pool(name="small", bufs=3) as small,
            tc.tile_pool(name="dg", bufs=2) as dgp,
            tc.tile_pool(name="xgt", bufs=2) as xgtp,
            tc.tile_pool(name="ysb", bufs=2) as ysbp,
            tc.tile_pool(name="pxio", bufs=2, space="PSUM") as pxio,
            tc.tile_pool(name="pdiag", bufs=2, space="PSUM") as pdiag,
            tc.tile_pool(name="psmall", bufs=2, space="PSUM") as psmall,
            tc.tile_pool(name="py", bufs=2, space="PSUM") as pyp,
        ):
            # ---- constants ----
            zero_t = consts.tile([128, 1], f32)
            nc.gpsimd.memset(zero_t[:], 0.0)
            nc.const_aps.aps[(f32, 0.0)] = zero_t[:]

            ident_f = consts.tile([128, 128], f32)
            make_identity(nc, ident_f[:])
            ident_m = consts.tile([128, 128], mm_dt)
            make_identity(nc, ident_m[:])

            cw_t = consts.tile([128, KSZ], f32)   # conv_w per-channel, dup 2x
            nc.sync.dma_start(cw_t[0:64, :], cw_d[:, 0, :])
            nc.sync.dma_start(cw_t[64:128, :], cw_d[:, 0, :])
            cb_t = consts.tile([128, 1], f32)
            nc.sync.dma_start(cb_t[0:64, :], cb_d[:, None])
            nc.sync.dma_start(cb_t[64:128, :], cb_d[:, None])

            gwT = consts.tile([CONV_DIM, NM], f32)  # gate_w.T
            nc.sync.dma_start(gwT[:, :], gw_d[:].rearrange("m d -> d m"))
            gb_t = consts.tile([128, NM], f32)      # gate_b bcast over partitions
            nc.sync.dma_start(gb_t[:, :], gb_d[None, :].broadcast_to([128, NM]))

            # gbb = conv_b * rowsum(gate_w) + gate_b   [128, 8]
            ones63 = consts.tile([CONV_DIM, 1], f32)
            nc.vector.memset(ones63[:, :], 1.0)
            gsum_tile = psmall.tile([128, 512], f32, tag="psg")
            gsum_ps = gsum_tile[0:1, 0:NM]
            nc.tensor.matmul(gsum_ps, ones63[:], gwT[:], start=True, stop=True)
            gsum_sb = consts.tile([1, NM], f32)
            nc.vector.tensor_copy(gsum_sb[:], gsum_ps)
            gsum_b = consts.tile([128, NM], f32)
            nc.gpsimd.partition_broadcast(gsum_b[:], gsum_sb[:])
            gbb = consts.tile([128, NM], f32)
            nc.vector.scalar_tensor_tensor(
                gbb[:], gsum_b[:], cb_t[:], gb_t[:], AL.mult, AL.add
            )

            # expert weights, block-diag paired, duplicated on both 64-rows:
            # wblk[64*half + sp*32 + n, m*90 + sp*45 + o] = map_w[m, o, n]
            wblk_f = consts.tile([128, NM * 90], f32)
            nc.vector.memset(wblk_f[:, :], 0.0)
            for half in range(2):
                for sp in range(2):
                    for m in range(NM):
                        nc.sync.dma_start(
                            wblk_f[
                                64 * half + 32 * sp : 64 * half + 32 * sp + 32,
                                m * 90 + sp * 45 : m * 90 + sp * 45 + 45,
                            ],
                            mw_d[m].rearrange("o n -> n o"),
                        )
            wblk = consts.tile([128, NM * 90], mm_dt)
            nc.vector.tensor_copy(wblk[:], wblk_f[:])

            # bias weights (block-diag): mbp2[sp*8 + j, sp*45 + o] = map_b[j,o]
            mbp2_f = consts.tile([16, 90], f32)
            nc.vector.memset(mbp2_f[:, :], 0.0)
            for sp in range(2):
                nc.gpsimd.dma_start(
                    mbp2_f[sp * 8 : sp * 8 + 8, sp * 45 : sp * 45 + 45], mb_d[:, :]
                )
            mbp2 = consts.tile([16, 90], mm_dt)
            nc.vector.tensor_copy(mbp2[:], mbp2_f[:])
            ones90 = consts.tile([1, 90], mm_dt)
            nc.vector.memset(ones90[:, :], 1.0)

            for g in range(NT // G):
                # ---- per group: load + transpose + stats + xc ----
                xc_g = xcg.tile([128, G * SEQ], f32, tag="xc")
                xcb_g = xcg.tile([128, G * SEQ], mm_dt, tag="xcb")
                mean_g = xcg.tile([128, G], f32, tag="mean_g")
                std_g = xcg.tile([128, G], f32, tag="std_g")
                rstd_g = xcg.tile([128, G], f32, tag="rstd_g")
                for i in range(G):
                    t = g * G + i
                    xraw = xin.tile([128, SEQ], f32, tag="xraw")
                    xrv = xraw[:].rearrange("p (j h c) -> p j h c", j=4, h=2)
                    for h in range(2):
                        nc.sync.dma_start(
                            xrv[:, :, h],
                            x_d[2 * t + h].rearrange("(j p) c -> p j c", p=128),
                        )
                    xrv2 = xraw[:].rearrange("p (j hc) -> p j hc", j=4)
                    psx = pxio.tile([128, SEQ], f32, tag="pxio")
                    for j in range(4):
                        nc.tensor.transpose(
                            psx[:, j * 128 : (j + 1) * 128], xrv2[:, j], ident_f[:]
                        )
                    # stats straight off PSUM
                    s1 = small.tile([128, 1], f32, tag="s1")
                    nc.vector.tensor_reduce(s1[:], psx[:], axis=AX.X, op=AL.add)
                    mean = mean_g[:, i : i + 1]
                    nc.vector.tensor_scalar(mean, s1[:], inv_L, None, AL.mult)
                    xc = xc_g[:, i * SEQ : (i + 1) * SEQ]
                    nc.vector.tensor_scalar(xc, psx[:], mean, None, AL.subtract)
                    sq = xin.tile([128, SEQ], f32, tag="sq")
                    m2 = small.tile([128, 1], f32, tag="m2")
                    nc.scalar.activation(sq[:], xc, AF.Square, accum_out=m2[:])
                    # var = m2/L (E[xc]^2 ~ 1e-14, eps dwarfed: both dropped)
                    std = std_g[:, i : i + 1]
                    nc.scalar.activation(std, m2[:], AF.Sqrt, scale=inv_L)
                    rstd = rstd_g[:, i : i + 1]
                    nc.vector.reciprocal(rstd, std)
                    # bf16 copy for PE consumption, cols permuted to (q, s, n)
                    # so diag-T stationaries are contiguous 128-col slices
                    nc.scalar.copy(
                        xcb_g[:, i * SEQ : (i + 1) * SEQ].rearrange(
                            "p (q s n) -> p q s n", q=4, s=4, n=N_IN
                        ),
                        xc_g[:, i * SEQ : (i + 1) * SEQ].rearrange(
                            "p (n q s) -> p q s n", n=N_IN, q=4, s=4
                        ),
                    )

                # ---- conv for the whole group (raw: no rstd, no conv_b) ----
                cv_g = cvg.tile([128, G * CONV_DIM], f32, tag="cv")
                cvv = cv_g[:].rearrange("p (i d) -> p i d", i=G)
                xcv2 = xc_g[:].rearrange("p (i d k) -> p i d k", i=G, k=STRIDE)
                for k in range(KSZ):
                    src = xcv2[
                        :, :, (k // STRIDE) : (k // STRIDE) + CONV_DIM, k % STRIDE
                    ]
                    if k == 0:
                        nc.vector.tensor_scalar(
                            cvv, src, cw_t[:, 0:1], None, AL.mult
                        )
                    else:
                        nc.vector.scalar_tensor_tensor(
                            cvv, src, cw_t[:, k : k + 1], cvv, AL.mult, AL.add
                        )

                for i in range(G):
                    t = g * G + i
                    std = std_g[:, i : i + 1]
                    rstd = rstd_g[:, i : i + 1]
                    mean = mean_g[:, i : i + 1]
                    # ---- gate logits: cvT = cv^T @ diag(rstd), then @ gwT ----
                    Drs = small.tile([128, 128], f32, tag="Drs")
                    nc.vector.tensor_scalar(
                        Drs[:], ident_f[:], rstd, None, AL.mult
                    )
                    pgate = psmall.tile([128, 512], f32, tag="psg")
                    cps = pgate[0:CONV_DIM, 0:128]
                    nc.tensor.matmul(
                        cps, cvv[:, i], Drs[:], start=True, stop=True
                    )
                    cvT = small.tile([CONV_DIM, 128], f32, tag="cvT")
                    nc.scalar.copy(cvT[:], cps)
                    lps = pgate[:, 128 : 128 + NM]
                    nc.tensor.matmul(lps, cvT[:], gwT[:], start=True, stop=True)
                    lg = small.tile([128, NM], f32, tag="lg")
                    nc.vector.tensor_tensor(lg[:], lps, gbb[:], AL.add)

                    # ---- softmax over 8, top-2, renorm softmax -> g ----
                    E1 = small.tile([128, NM], f32, tag="E1")
                    se1 = small.tile([128, 1], f32, tag="se1")
                    nc.scalar.activation(E1[:], lg[:], AF.Exp, accum_out=se1[:])
                    r1 = small.tile([128, 1], f32, tag="r1")
                    nc.vector.reciprocal(r1[:], se1[:])
                    v = small.tile([128, NM], f32, tag="v")
                    nc.vector.tensor_scalar(v[:], E1[:], r1[:], None, AL.mult)
                    E2 = small.tile([128, NM], f32, tag="E2")
                    nc.scalar.activation(E2[:], v[:], AF.Exp)
                    m8 = small.tile([128, 8], f32, tag="m8")
                    nc.vector.max(m8[:], E2[:])
                    msk = small.tile([128, NM], f32, tag="msk")
                    nc.vector.tensor_scalar(
                        msk[:], E2[:], m8[:, 1:2], None, AL.is_ge
                    )
                    Em = small.tile([128, NM], f32, tag="Em")
                    se2 = small.tile([128, 1], f32, tag="se2")
                    nc.vector.scalar_tensor_tensor(
                        Em[:], E2[:], 1.0, msk[:], AL.bypass, AL.mult,
                        accum_out=se2[:],
                    )
                    r2 = small.tile([128, 1], f32, tag="r2")
                    nc.vector.reciprocal(r2[:], se2[:])
                    gt = small.tile([128, NM], f32, tag="gt")
                    nc.vector.tensor_scalar(gt[:], Em[:], r2[:], None, AL.mult)

                    # gsm2 = [g*std twice | mean]  [128, 17]
                    gsm2 = small.tile([128, 17], f32, tag="gsm2")
                    nc.vector.tensor_scalar(
                        gsm2[:, 0:NM], gt[:], std, None, AL.mult
                    )
                    nc.scalar.copy(gsm2[:, 8:16], gsm2[:, 0:NM])
                    nc.vector.tensor_copy(gsm2[:, 16:17], mean)
                    gps = pgate[0:16, 256:384]
                    nc.tensor.transpose(gps, gsm2[:, 0:16], ident_f[:])
                    mps = pgate[0:1, 384:512]
                    nc.tensor.transpose(mps, gsm2[:, 16:17], ident_f[:])
                    gsmT2 = small.tile([16, 512], mm_dt, tag="gsmT2")
                    mT = small.tile([1, 512], mm_dt, tag="mT")
                    for q in range(4):
                        if q % 2 == 0:
                            nc.vector.tensor_copy(
                                gsmT2[:, q * 128 : (q + 1) * 128], gps
                            )
                            nc.scalar.copy(mT[:, q * 128 : (q + 1) * 128], mps)
                        else:
                            nc.scalar.copy(
                                gsmT2[:, q * 128 : (q + 1) * 128], gps
                            )
                            nc.vector.tensor_copy(
                                mT[:, q * 128 : (q + 1) * 128], mps
                            )

                    # ---- D_m diagonal gate matrices (bf16) ----
                    Dall = dgp.tile([128, NM * 128], mm_dt, tag="D")
                    for m in range(NM):
                        dst = Dall[:, m * 128 : (m + 1) * 128]
                        if m % 2 == 0:
                            nc.vector.tensor_scalar(
                                dst, ident_m[:], gt[:, m : m + 1], None, AL.mult
                            )
                        else:
                            nc.scalar.mul(dst, ident_m[:], gt[:, m : m + 1])

                    # ---- diag-scaled transposes: xgt_m = xcb_q^T @ D_m ----
                    # xcb cols ordered (q, s, n): block q is contiguous 128
                    xcb_t = xcb_g[:, i * SEQ : (i + 1) * SEQ]
                    xgt = xgtp.tile([128, NM * SEQ], mm_dt, tag="xgt")
                    for m in range(NM):
                        ptp = pdiag.tile([128, SEQ], f32, tag="ptp")
                        for q in range(4):
                            nc.tensor.matmul(
                                ptp[:, q * 128 : (q + 1) * 128],
                                xcb_t[:, q * 128 : (q + 1) * 128],
                                Dall[:, m * 128 : (m + 1) * 128],
                                start=True,
                                stop=True,
                            )
                        dst = xgt[:, m * SEQ : (m + 1) * SEQ]
                        if m % 2 == 0:
                            nc.vector.tensor_copy(dst, ptp[:])
                        else:
                            nc.scalar.copy(dst, ptp[:])

                    # ---- expert matmuls: per pair-group, accumulate over m ----
                    ysb = ysbp.tile([90, 1024], f32, tag="ysb")
                    for pg in range(2):
                        yps = pyp.tile([90, 512], f32, tag="yps")
                        for m in range(NM):
                            nc.tensor.matmul(
                                yps[:],
                                wblk[64 * pg : 64 * (pg + 1), m * 90 : (m + 1) * 90],
                                xgt[:, m * SEQ : (m + 1) * SEQ][
                                    64 * pg : 64 * (pg + 1), :
                                ],
                                start=(m == 0),
                                stop=False,
                            )
                        nc.tensor.matmul(
                            yps[:], mbp2[:], gsmT2[:], start=False, stop=False
                        )
                        nc.tensor.matmul(
                            yps[:], ones90[:], mT[:], start=False, stop=True
                        )
                        # ysb cols laid out (q, pg, h, c) so the store AP merges
                        dst = ysb[:].rearrange(
                            "p (q pg h c) -> p q pg h c", q=4, pg=2, h=2
                        )[:, :, pg]
                        src = yps[:].rearrange("p (q h c) -> p q h c", q=4, h=2)
                        if pg == 0:
                            nc.vector.tensor_copy(dst, src)
                        else:
                            nc.scalar.copy(dst, src)

                    # ---- store: p = 16o + 4q + 2pg + sp ----
                    ysv = ysb[:].rearrange(
                        "p (qpg h c) -> p qpg h c", qpg=8, h=2
                    )
                    for h in range(2):
                        for sp in range(2):
                            nc.gpsimd.dma_start(
                                out_d[2 * t + h].rearrange(
                                    "(o qpg sp) c -> o qpg sp c",
                                    qpg=8, sp=2,
                                )[:, :, sp],
                                ysv[45 * sp : 45 * sp + 45, :, h],
                            )

    nc.compile()
    return nc


def _get_program(mm_dt_name):
    key = mm_dt_name
    if key not in _CACHE:
        _CACHE[key] = _build_program(key)
    return _CACHE[key]


def kernel(x, conv_w, conv_b, gate_w, gate_b, map_w, map_b, _mm_dt="bfloat16",
           _trace=False):
    from concourse.bass_utils import run_bass_kernel_spmd

    nc = _get_program(_mm_dt)
    x = np.ascontiguousarray(np.asarray(x, dtype=np.float32))
    params = dict(
        conv_w=np.ascontiguousarray(np.asarray(conv_w, np.float32)),
        conv_b=np.ascontiguousarray(np.asarray(conv_b, np.float32)),
        gate_w=np.ascontiguousarray(np.asarray(gate_w, np.float32)),
        gate_b=np.ascontiguousarray(np.asarray(gate_b, np.float32)),
        map_w=np.ascontiguousarray(np.asarray(map_w, np.float32)),
        map_b=np.ascontiguousarray(np.asarray(map_b, np.float32)),
    )
    in_maps = [
        dict(x=x[i * BPC : (i + 1) * BPC], **params) for i in range(NCORES)
    ]
    res = run_bass_kernel_spmd(
        nc, in_maps, core_ids=list(range(NCORES)), trace=_trace
    )
    out = np.concatenate([res.results[i]["out"] for i in range(NCORES)], axis=0)
    if _trace:
        return out, res
    return out
